# revision 16
# baseline (speedup 1.0000x reference)
"""Trainium2 Bass kernel for Conv1d(stride 512, K=3072) -> GRU encoder.

Sharding: data-parallel over batch (B=8) across the 8 NeuronCores; GRU
weights replicated; the sequential time loop runs fully on-device per core.

Per core (batch element b):
  - conv as 6 shifted frame-matmuls, contraction (Cin*512=2048) in 16
    K-chunks of 128, float32r (1 cycle/row moving operand).
  - gi = convT @ w_ih^T + (w_ih@conv_b + b_ih [+ b_hh for r,z]) precomputed
    for all timesteps, laid out [128, T*12] so step t reads [128, 12].
  - GRU scan: per step 48 bf16 matmuls (W_hh stationary tiles [128,128],
    h as [128,1] moving operand) -> gh^T in PSUM [128, 12] where column m
    holds neurons [128m, 128m+128); gates on DVE/ACT; h state kept fp32 in
    the ys buffer, with a bf16 copy feeding the next step's matmuls.
"""
import os
import sys

sys.path.insert(0, "/opt/trn_rl_repo")

import numpy as np
import ml_dtypes

import concourse.bass as bass
import concourse.tile as tile
from concourse import mybir
from concourse.bass_utils import run_bass_kernel_spmd
from concourse.vector_clock import ScopedClock

# ---------------------------------------------------------------- constants
B, CIN, L = 8, 4, 524288
KERNEL, STRIDE = 3072, 512
COUT, H = 512, 512
G3 = 3 * H  # 1536
NF = L // STRIDE  # 1024 frames
T = (L - KERNEL) // STRIDE + 1  # 1019
NK = (CIN * STRIDE) // 128  # 16 contraction chunks
NJ = KERNEL // STRIDE  # 6 shifts
NM = COUT // 128  # 4 conv out tiles
NG = G3 // 128  # 12 gate tiles
NH = H // 128  # 4 h chunks

F32 = mybir.dt.float32
F32R = mybir.dt.float32r
BF16 = mybir.dt.bfloat16

_CACHE = {}


def _patch_tile_drain():
    """This walrus build rejects >2 (CTRL: >1) sync waits per instruction;
    spread the Tile kernel-tail drain waits over single-wait SP nops."""
    if getattr(tile.TileContext, "_drain_patched", False):
        return

    def _drain_and_barrier(self, tick_clock, wait_clock):
        nop_inst = self.nc.sync.nop()
        wait_clock.add_sem_waits(
            nop_inst.ins, ScopedClock({None: tick_clock.global_clock})
        )
        si = nop_inst.ins.sync_info
        if si is not None and si.on_wait and len(si.on_wait) > 1:
            waits = list(si.on_wait)
            updates = list(si.on_update)
            nop_inst.ins.sync_info = mybir.SyncInfo(
                on_wait=waits[:1], on_update=updates
            )
            for i in range(1, len(waits)):
                extra = self.nc.sync.nop()
                extra.ins.sync_info = mybir.SyncInfo(
                    on_wait=waits[i : i + 1], on_update=[]
                )
        self.nc.sync.drain()
        self.nc.all_engine_barrier()
        assert self.sems is not None
        popped = self.nc._tile_sem_poison_stack.pop()
        assert popped is self._sem_poison
        self.nc.clear_and_free_semaphores(list(self.sems.allocated().values()))
        self.nc.all_engine_barrier()

    tile.TileContext._drain_and_barrier = _drain_and_barrier
    tile.TileContext._drain_patched = True


_CTRL_OPS = {"NoOp", "Drain"}


def _split_excess_waits(nc):
    """This walrus build allows at most 2 sync waits per instruction (1 for
    CTRL ops like NoOp/Drain). Hoist excess waits onto preceding same-engine
    single-wait nops."""
    for fn in nc.m.functions:
        for blk in fn.blocks:
            changed = False
            out = []
            for inst in blk.instructions:
                si = inst.sync_info
                limit = 1
                if si is not None and si.on_wait and len(si.on_wait) > limit:
                    waits = list(si.on_wait)
                    extra, keep = waits[:-limit], waits[-limit:]
                    for k, w in enumerate(extra):
                        out.append(
                            mybir.InstNoOp(
                                name=f"{inst.name}_ws{k}",
                                engine=inst.engine,
                                sync_info=mybir.SyncInfo(on_wait=[w], on_update=[]),
                            )
                        )
                    inst.sync_info = mybir.SyncInfo(
                        on_wait=keep, on_update=list(si.on_update)
                    )
                    changed = True
                out.append(inst)
            if changed:
                blk.instructions = out


def build_program(t_steps=T):
    """Build the single-core Bass program (same program runs on all 8 cores
    with per-core input data)."""
    _patch_tile_drain()
    nc = bass.Bass("TRN2", target_bir_lowering=False, debug=False)

    x_in = nc.dram_tensor("x", [CIN, STRIDE, NF], F32R, kind="ExternalInput").ap()
    h0_in = nc.dram_tensor("h0", [128, NH], F32, kind="ExternalInput").ap()
    wconv_in = nc.dram_tensor(
        "wconv", [128, NK * NM * NJ * 128], F32R, kind="ExternalInput"
    ).ap()
    wih_in = nc.dram_tensor("wih", [128, NG * NH * 128], F32R, kind="ExternalInput").ap()
    whh_in = nc.dram_tensor(
        "whh", [128, NG * NH * 128], BF16, kind="ExternalInput"
    ).ap()
    gibias_in = nc.dram_tensor("gibias", [128, NG], F32, kind="ExternalInput").ap()
    bhhn_in = nc.dram_tensor("bhhn", [128, NH], F32, kind="ExternalInput").ap()
    ys_out = nc.dram_tensor(
        "ys", [128, NH * (t_steps + 1)], F32, kind="ExternalOutput"
    ).ap()

    with tile.TileContext(nc) as tc:
        with (
            tc.tile_pool(name="consts", bufs=1) as cpool,
            tc.tile_pool(name="wih", bufs=1) as wihpool,
            tc.tile_pool(name="whh", bufs=1) as whhpool,
            tc.tile_pool(name="convT", bufs=1) as ctpool,
            tc.tile_pool(name="gi", bufs=1) as gipool,
            tc.tile_pool(name="ys", bufs=1) as yspool,
            tc.tile_pool(name="hbf", bufs=1) as hpool,
            tc.tile_pool(name="gates", bufs=2) as gpool,
        ):
            # ---- persistent tiles
            gibias = cpool.tile([128, NG], F32, tag="gibias")
            nc.sync.dma_start(gibias[:], gibias_in[:])
            bhhn = cpool.tile([128, NH], F32, tag="bhhn")
            nc.sync.dma_start(bhhn[:], bhhn_in[:])
            wih = wihpool.tile([128, NG * NH * 128], F32R)
            nc.sync.dma_start(wih[:], wih_in[:])
            whh = whhpool.tile([128, NG * NH * 128], BF16)
            nc.sync.dma_start(whh[:], whh_in[:])
            convT = ctpool.tile([128, NM * NF], F32R)
            gi = gipool.tile([128, t_steps * NG], F32)
            ys = yspool.tile([128, NH * (t_steps + 1)], F32)
            h16 = hpool.tile([128, NH], BF16)

            nc.sync.dma_start(ys[:, 0:NH], h0_in[:])
            nc.vector.tensor_copy(h16[:], ys[:, 0:NH])

            # ---- conv phase: psum[m, tc] accumulates over all (kc, j)
            # moving free dims must be even for fp32r; pad odd chunks by one
            # (padded frames are zeroed; padded outputs are discarded)
            if t_steps > 512:
                nts = [512, t_steps - 512]
            else:
                nts = [t_steps]
            mm_nts = [n + (n % 2) for n in nts]
            NFP = NF + 8
            conv_scope = tc.tile_pool(name="inp", bufs=2)
            inppool = conv_scope.__enter__()
            wconv_scope = tc.tile_pool(name="wconv", bufs=2)
            wcpool = wconv_scope.__enter__()
            psconv_scope = tc.tile_pool(name="psconv", bufs=1, space="PSUM")
            psconv = psconv_scope.__enter__()
            ps_conv = [
                [psconv.tile([128, 512], F32, tag=f"psc{m}_{tci}", name=f"psc{m}_{tci}") for tci in range(len(nts))]
                for m in range(NM)
            ]
            for kc in range(NK):
                inp_k = inppool.tile([128, NFP], F32R, tag="inp")
                nc.vector.memset(inp_k[:, NF:NFP].bitcast(mybir.dt.uint32), 0)
                # x[c] viewed [NF, 512]; partitions = s-chunk, free = frame
                c, s1 = kc // 4, kc % 4
                nc.sync.dma_start(
                    inp_k[:, :NF], x_in[c, 128 * s1 : 128 * (s1 + 1), :]
                )
                wconv_k = wcpool.tile([128, NM * NJ * 128], F32R, tag="wconv")
                nc.sync.dma_start(
                    wconv_k[:],
                    wconv_in[:, kc * NM * NJ * 128 : (kc + 1) * NM * NJ * 128],
                )
                for m in range(NM):
                    for j in range(NJ):
                        lhsT = wconv_k[:, (m * NJ + j) * 128 : (m * NJ + j + 1) * 128]
                        for tci, nt in enumerate(mm_nts):
                            nc.tensor.matmul(
                                ps_conv[m][tci][:, :nt],
                                lhsT,
                                inp_k[:, j + 512 * tci : j + 512 * tci + nt],
                                start=(kc == 0 and j == 0),
                                stop=(kc == NK - 1 and j == NJ - 1),
                            )
            # evacuate conv psum -> convT (column layout: m*NF + t)
            for m in range(NM):
                for tci, nt in enumerate(nts):
                    nc.vector.tensor_copy(
                        convT[:, m * NF + 512 * tci : m * NF + 512 * tci + nt],
                        ps_conv[m][tci][:, :nt],
                    )

            psconv_scope.__exit__(None, None, None)
            wconv_scope.__exit__(None, None, None)
            conv_scope.__exit__(None, None, None)

            # ---- gi phase: gi^T = wih @ convT (+bias), layout [128, t*NG+g]
            gi3 = gi[:].rearrange("p (t g) -> p t g", g=NG)
            psgi_scope = tc.tile_pool(name="psgi", bufs=2, space="PSUM")
            psgi = psgi_scope.__enter__()
            for mg in range(NG):
                for tci, nt in enumerate(nts):
                    ntm = mm_nts[tci]
                    ps = psgi.tile([128, 512], F32, tag="psgi")
                    for kc in range(NH):
                        nc.tensor.matmul(
                            ps[:, :ntm],
                            wih[:, (mg * NH + kc) * 128 : (mg * NH + kc + 1) * 128],
                            convT[:, kc * NF + 512 * tci : kc * NF + 512 * tci + ntm],
                            start=(kc == 0),
                            stop=(kc == NH - 1),
                        )
                    nc.vector.tensor_scalar(
                        gi3[:, 512 * tci : 512 * tci + nt, mg],
                        ps[:, :nt],
                        gibias[:, mg : mg + 1],
                        None,
                        mybir.AluOpType.add,
                    )

            psgi_scope.__exit__(None, None, None)

            # ---- GRU scan
            with tc.tile_pool(name="psgh", bufs=2, space="PSUM") as psgh:
                for t in range(t_steps):
                    gh_rz = psgh.tile([128, 8], F32, tag="ghrz")
                    gh_n = psgh.tile([128, NH], F32, tag="ghn")
                    prerz = psgh.tile([128, 8], F32, tag="prz")
                    npre = psgh.tile([128, 4], F32, tag="npr")
                    for mg in range(NG):
                        dst = (
                            gh_rz[:, mg : mg + 1]
                            if mg < 8
                            else gh_n[:, mg - 8 : mg - 7]
                        )
                        for kc in range(NH):
                            nc.tensor.matmul(
                                dst,
                                whh[:, (mg * NH + kc) * 128 : (mg * NH + kc + 1) * 128],
                                h16[:, kc : kc + 1],
                                start=(kc == 0),
                                stop=(kc == NH - 1),
                            )
                    nc.vector.tensor_add(prerz[:], gh_rz[:], gi3[:, t, 0:8])
                    rz = gpool.tile([128, 8], F32, tag="rz")
                    nc.scalar.activation(
                        rz[:], prerz[:], mybir.ActivationFunctionType.Sigmoid
                    )
                    hn = gpool.tile([128, 4], F32, tag="hn")
                    nc.vector.tensor_add(hn[:], gh_n[:], bhhn[:])
                    t1 = gpool.tile([128, 4], F32, tag="t1")
                    nc.vector.tensor_mul(t1[:], rz[:, 0:4], hn[:])
                    nc.vector.tensor_add(npre[:], t1[:], gi3[:, t, 8:12])
                    nn_ = gpool.tile([128, 4], F32, tag="nn")
                    nc.scalar.activation(
                        nn_[:], npre[:], mybir.ActivationFunctionType.Tanh
                    )
                    # off-critical-path helpers (overlap with tanh):
                    c1 = gpool.tile([128, 4], F32, tag="c1")  # 1 - z
                    nc.vector.tensor_scalar(
                        c1[:],
                        rz[:, 4:8],
                        -1.0,
                        1.0,
                        mybir.AluOpType.mult,
                        mybir.AluOpType.add,
                    )
                    c2 = gpool.tile([128, 4], F32, tag="c2")  # z * h
                    nc.vector.tensor_mul(
                        c2[:], rz[:, 4:8], ys[:, NH * t : NH * (t + 1)]
                    )
                    t2 = gpool.tile([128, 4], F32, tag="t2")
                    nc.vector.tensor_mul(t2[:], c1[:], nn_[:])
                    # critical path: bf16 state for the next step's matmuls
                    nc.vector.tensor_add(h16[:], t2[:], c2[:])
                    # off critical path: fp32 state archive (read by c2 at t+1)
                    nc.vector.tensor_add(
                        ys[:, NH * (t + 1) : NH * (t + 2)], t2[:], c2[:]
                    )

            nc.sync.dma_start(ys_out[:], ys[:])

    _split_excess_waits(nc)
    return nc


def _prep_weights(conv_w, conv_b, w_ih, w_hh, b_ih, b_hh):
    """Host-side weight layouts (see build_program for the tile maps)."""
    f32 = np.float32
    # wconv[p, ((kc*NM + m)*NJ + j)*128 + co'] = conv_w[128m+co', c, 512j+128s1+p]
    A = conv_w.reshape(NM, 128, CIN, NJ, 4, 128)  # [m, co', c, j, s1, p]
    wconv = np.ascontiguousarray(
        A.transpose(2, 4, 0, 3, 5, 1), dtype=f32
    )  # [c, s1, m, j, p, co']
    wconv = wconv.reshape(NK, NM, NJ, 128, 128).transpose(3, 0, 1, 2, 4).reshape(
        128, NK * NM * NJ * 128
    )
    wconv = np.ascontiguousarray(wconv, dtype=f32)

    # wih[p, (mg*NH + kc)*128 + n'] = w_ih[128mg+n', 128kc+p]
    Aih = w_ih.reshape(NG, 128, NH, 128)  # [mg, n', kc, p]
    wih = np.ascontiguousarray(
        Aih.transpose(3, 0, 2, 1).reshape(128, NG * NH * 128), dtype=f32
    )
    Ahh = w_hh.reshape(NG, 128, NH, 128)
    whh = np.ascontiguousarray(
        Ahh.transpose(3, 0, 2, 1).reshape(128, NG * NH * 128)
    ).astype(ml_dtypes.bfloat16)

    q = (w_ih.astype(np.float64) @ conv_b.astype(np.float64)).astype(f32)
    q = q + b_ih
    q[: 2 * H] = q[: 2 * H] + b_hh[: 2 * H]
    gibias = np.ascontiguousarray(q.reshape(NG, 128).T, dtype=f32)
    bhhn = np.ascontiguousarray(b_hh[2 * H :].reshape(NH, 128).T, dtype=f32)
    return wconv, wih, whh, gibias, bhhn


def kernel(input, hidden, conv_w, conv_b, w_ih, w_hh, b_ih, b_hh, _trace=False):
    input = np.asarray(input, dtype=np.float32)
    hidden = np.asarray(hidden, dtype=np.float32)
    conv_w = np.asarray(conv_w, dtype=np.float32)
    conv_b = np.asarray(conv_b, dtype=np.float32)
    w_ih = np.asarray(w_ih, dtype=np.float32)
    w_hh = np.asarray(w_hh, dtype=np.float32)
    b_ih = np.asarray(b_ih, dtype=np.float32)
    b_hh = np.asarray(b_hh, dtype=np.float32)

    if ("prog", T) not in _CACHE:
        _CACHE[("prog", T)] = build_program(T)
    nc = _CACHE[("prog", T)]

    wconv, wih, whh, gibias, bhhn = _prep_weights(
        conv_w, conv_b, w_ih, w_hh, b_ih, b_hh
    )
    in_maps = []
    for b in range(B):
        xb = input[b].reshape(CIN, NF, STRIDE).transpose(0, 2, 1)
        in_maps.append(
            {
                "x": np.ascontiguousarray(xb),
                "h0": np.ascontiguousarray(hidden[0, b].reshape(NH, 128).T),
                "wconv": wconv,
                "wih": wih,
                "whh": whh,
                "gibias": gibias,
                "bhhn": bhhn,
            }
        )
    res = run_bass_kernel_spmd(nc, in_maps, list(range(B)))

    ys_full = np.empty((T, B, H), dtype=np.float32)
    for b in range(B):
        yb = res.results[b]["ys"]  # [128, NH*(T+1)]
        yb = yb.reshape(128, T + 1, NH)[:, 1:, :]  # drop h0
        ys_full[:, b, :] = yb.transpose(1, 2, 0).reshape(T, H)
    hT = ys_full[-1]  # [B, H]
    out1 = ys_full.reshape(1, T * B, H)
    out2 = hT[None]
    return out1, out2


# revision 17
# speedup vs baseline: 3.3592x; 3.3592x over previous
"""Trainium2 Bass kernel for Conv1d(stride 512, K=3072) -> GRU encoder.

Sharding: data-parallel over batch (B=8) across the 8 NeuronCores; GRU
weights replicated; the sequential time loop runs fully on-device per core.

Per core (batch element b):
  - conv as 6 shifted frame-matmuls, contraction (Cin*512=2048) in 16
    K-chunks of 128, float32r (1 cycle/row moving operand).
  - gi = convT @ w_ih^T + (w_ih@conv_b + b_ih [+ b_hh for r,z]) precomputed
    for all timesteps, laid out [128, T*12] so step t reads [128, 12].
  - GRU scan: per step 48 bf16 matmuls (W_hh stationary tiles [128,128],
    h as [128,1] moving operand) -> gh^T in PSUM [128, 12] where column m
    holds neurons [128m, 128m+128); gates on DVE/ACT; h state kept fp32 in
    the ys buffer, with a bf16 copy feeding the next step's matmuls.
"""
import os
import sys

sys.path.insert(0, "/opt/trn_rl_repo")

import numpy as np
import ml_dtypes

import concourse.bass as bass
import concourse.tile as tile
from concourse import mybir
from concourse.bass_utils import run_bass_kernel_spmd
from concourse.vector_clock import ScopedClock

# ---------------------------------------------------------------- constants
B, CIN, L = 8, 4, 524288
KERNEL, STRIDE = 3072, 512
COUT, H = 512, 512
G3 = 3 * H  # 1536
NF = L // STRIDE  # 1024 frames
T = (L - KERNEL) // STRIDE + 1  # 1019
NK = (CIN * STRIDE) // 128  # 16 contraction chunks
NJ = KERNEL // STRIDE  # 6 shifts
NM = COUT // 128  # 4 conv out tiles
NG = G3 // 128  # 12 gate tiles
NH = H // 128  # 4 h chunks

F32 = mybir.dt.float32
F32R = mybir.dt.float32r
BF16 = mybir.dt.bfloat16

_CACHE = {}


def _patch_tile_drain():
    """This walrus build rejects >2 (CTRL: >1) sync waits per instruction;
    spread the Tile kernel-tail drain waits over single-wait SP nops."""
    if getattr(tile.TileContext, "_drain_patched", False):
        return

    def _drain_and_barrier(self, tick_clock, wait_clock):
        nop_inst = self.nc.sync.nop()
        wait_clock.add_sem_waits(
            nop_inst.ins, ScopedClock({None: tick_clock.global_clock})
        )
        si = nop_inst.ins.sync_info
        if si is not None and si.on_wait and len(si.on_wait) > 1:
            waits = list(si.on_wait)
            updates = list(si.on_update)
            nop_inst.ins.sync_info = mybir.SyncInfo(
                on_wait=waits[:1], on_update=updates
            )
            for i in range(1, len(waits)):
                extra = self.nc.sync.nop()
                extra.ins.sync_info = mybir.SyncInfo(
                    on_wait=waits[i : i + 1], on_update=[]
                )
        self.nc.sync.drain()
        self.nc.all_engine_barrier()
        assert self.sems is not None
        popped = self.nc._tile_sem_poison_stack.pop()
        assert popped is self._sem_poison
        self.nc.clear_and_free_semaphores(list(self.sems.allocated().values()))
        self.nc.all_engine_barrier()

    tile.TileContext._drain_and_barrier = _drain_and_barrier
    tile.TileContext._drain_patched = True


_CTRL_OPS = {"NoOp", "Drain"}


def _split_excess_waits(nc):
    """This walrus build allows at most 2 sync waits per instruction (1 for
    CTRL ops like NoOp/Drain). Hoist excess waits onto preceding same-engine
    single-wait nops."""
    for fn in nc.m.functions:
        for blk in fn.blocks:
            changed = False
            out = []
            for inst in blk.instructions:
                si = inst.sync_info
                limit = 1
                if si is not None and si.on_wait and len(si.on_wait) > limit:
                    waits = list(si.on_wait)
                    extra, keep = waits[:-limit], waits[-limit:]
                    for k, w in enumerate(extra):
                        out.append(
                            mybir.InstNoOp(
                                name=f"{inst.name}_ws{k}",
                                engine=inst.engine,
                                sync_info=mybir.SyncInfo(on_wait=[w], on_update=[]),
                            )
                        )
                    inst.sync_info = mybir.SyncInfo(
                        on_wait=keep, on_update=list(si.on_update)
                    )
                    changed = True
                out.append(inst)
            if changed:
                blk.instructions = out


def build_program(t_steps=T):
    """Build the single-core Bass program (same program runs on all 8 cores
    with per-core input data)."""
    _patch_tile_drain()
    nc = bass.Bass("TRN2", target_bir_lowering=False, debug=False)

    x_in = nc.dram_tensor("x", [CIN, STRIDE, NF], F32R, kind="ExternalInput").ap()
    h0_in = nc.dram_tensor("h0", [128, NH], F32, kind="ExternalInput").ap()
    wconv_in = nc.dram_tensor(
        "wconv", [128, NK * NM * NJ * 128], F32R, kind="ExternalInput"
    ).ap()
    wih_in = nc.dram_tensor("wih", [128, NG * NH * 128], F32R, kind="ExternalInput").ap()
    whh_in = nc.dram_tensor(
        "whh", [128, NG * NH * 128], BF16, kind="ExternalInput"
    ).ap()
    gibias_in = nc.dram_tensor("gibias", [128, NG], F32, kind="ExternalInput").ap()
    bhhn_in = nc.dram_tensor("bhhn", [128, NH], F32, kind="ExternalInput").ap()
    ys_out = nc.dram_tensor(
        "ys", [128, NH * (t_steps + 1)], F32, kind="ExternalOutput"
    ).ap()

    with tile.TileContext(nc) as tc:
        with (
            tc.tile_pool(name="consts", bufs=1) as cpool,
            tc.tile_pool(name="wih", bufs=1) as wihpool,
            tc.tile_pool(name="whh", bufs=1) as whhpool,
            tc.tile_pool(name="convT", bufs=1) as ctpool,
            tc.tile_pool(name="gi", bufs=1) as gipool,
            tc.tile_pool(name="ys", bufs=1) as yspool,
            tc.tile_pool(name="hbf", bufs=1) as hpool,
            tc.tile_pool(name="gates", bufs=2) as gpool,
        ):
            # ---- persistent tiles
            gibias = cpool.tile([128, NG], F32, tag="gibias")
            nc.sync.dma_start(gibias[:], gibias_in[:])
            bhhn = cpool.tile([128, NH], F32, tag="bhhn")
            nc.sync.dma_start(bhhn[:], bhhn_in[:])
            wih = wihpool.tile([128, NG * NH * 128], F32R)
            nc.sync.dma_start(wih[:], wih_in[:])
            whh = whhpool.tile([128, NG * NH * 128], BF16)
            nc.sync.dma_start(whh[:], whh_in[:])
            convT = ctpool.tile([128, NM * NF], F32R)
            gi = gipool.tile([128, t_steps * NG], F32)
            ys = yspool.tile([128, NH * (t_steps + 1)], F32)
            h16 = hpool.tile([128, NH], BF16)

            nc.sync.dma_start(ys[:, 0:NH], h0_in[:])
            nc.vector.tensor_copy(h16[:], ys[:, 0:NH])

            # ---- conv phase: psum[m, tc] accumulates over all (kc, j)
            # moving free dims must be even for fp32r; pad odd chunks by one
            # (padded frames are zeroed; padded outputs are discarded)
            if t_steps > 512:
                nts = [512, t_steps - 512]
            else:
                nts = [t_steps]
            mm_nts = [n + (n % 2) for n in nts]
            NFP = NF + 8
            conv_scope = tc.tile_pool(name="inp", bufs=2)
            inppool = conv_scope.__enter__()
            wconv_scope = tc.tile_pool(name="wconv", bufs=2)
            wcpool = wconv_scope.__enter__()
            psconv_scope = tc.tile_pool(name="psconv", bufs=1, space="PSUM")
            psconv = psconv_scope.__enter__()
            ps_conv = [
                [psconv.tile([128, 512], F32, tag=f"psc{m}_{tci}", name=f"psc{m}_{tci}") for tci in range(len(nts))]
                for m in range(NM)
            ]
            for kc in range(NK):
                inp_k = inppool.tile([128, NFP], F32R, tag="inp")
                nc.vector.memset(inp_k[:, NF:NFP].bitcast(mybir.dt.uint32), 0)
                # x[c] viewed [NF, 512]; partitions = s-chunk, free = frame
                c, s1 = kc // 4, kc % 4
                nc.sync.dma_start(
                    inp_k[:, :NF], x_in[c, 128 * s1 : 128 * (s1 + 1), :]
                )
                wconv_k = wcpool.tile([128, NM * NJ * 128], F32R, tag="wconv")
                nc.sync.dma_start(
                    wconv_k[:],
                    wconv_in[:, kc * NM * NJ * 128 : (kc + 1) * NM * NJ * 128],
                )
                for m in range(NM):
                    for j in range(NJ):
                        lhsT = wconv_k[:, (m * NJ + j) * 128 : (m * NJ + j + 1) * 128]
                        for tci, nt in enumerate(mm_nts):
                            nc.tensor.matmul(
                                ps_conv[m][tci][:, :nt],
                                lhsT,
                                inp_k[:, j + 512 * tci : j + 512 * tci + nt],
                                start=(kc == 0 and j == 0),
                                stop=(kc == NK - 1 and j == NJ - 1),
                            )
            # evacuate conv psum -> convT (column layout: m*NF + t)
            for m in range(NM):
                for tci, nt in enumerate(nts):
                    nc.vector.tensor_copy(
                        convT[:, m * NF + 512 * tci : m * NF + 512 * tci + nt],
                        ps_conv[m][tci][:, :nt],
                    )

            psconv_scope.__exit__(None, None, None)
            wconv_scope.__exit__(None, None, None)
            conv_scope.__exit__(None, None, None)

            # ---- gi phase: gi^T = wih @ convT (+bias), layout [128, t*NG+g]
            gi3 = gi[:].rearrange("p (t g) -> p t g", g=NG)
            psgi_scope = tc.tile_pool(name="psgi", bufs=2, space="PSUM")
            psgi = psgi_scope.__enter__()
            for mg in range(NG):
                for tci, nt in enumerate(nts):
                    ntm = mm_nts[tci]
                    ps = psgi.tile([128, 512], F32, tag="psgi")
                    for kc in range(NH):
                        nc.tensor.matmul(
                            ps[:, :ntm],
                            wih[:, (mg * NH + kc) * 128 : (mg * NH + kc + 1) * 128],
                            convT[:, kc * NF + 512 * tci : kc * NF + 512 * tci + ntm],
                            start=(kc == 0),
                            stop=(kc == NH - 1),
                        )
                    nc.vector.tensor_scalar(
                        gi3[:, 512 * tci : 512 * tci + nt, mg],
                        ps[:, :nt],
                        gibias[:, mg : mg + 1],
                        None,
                        mybir.AluOpType.add,
                    )

            psgi_scope.__exit__(None, None, None)

            # ---- GRU scan
            with tc.tile_pool(name="psgh", bufs=2, space="PSUM") as psgh:
                for t in range(t_steps):
                    gh_rz = psgh.tile([128, 8], F32, tag="ghrz")
                    gh_n = psgh.tile([128, NH], F32, tag="ghn")
                    prerz = psgh.tile([128, 8], F32, tag="prz")
                    npre = psgh.tile([128, 4], F32, tag="npr")
                    for mg in range(NG):
                        dst = (
                            gh_rz[:, mg : mg + 1]
                            if mg < 8
                            else gh_n[:, mg - 8 : mg - 7]
                        )
                        for kc in range(NH):
                            nc.tensor.matmul(
                                dst,
                                whh[:, (mg * NH + kc) * 128 : (mg * NH + kc + 1) * 128],
                                h16[:, kc : kc + 1],
                                start=(kc == 0),
                                stop=(kc == NH - 1),
                            )
                    nc.vector.tensor_add(prerz[:], gh_rz[:], gi3[:, t, 0:8])
                    rz = gpool.tile([128, 8], F32, tag="rz")
                    nc.scalar.activation(
                        rz[:], prerz[:], mybir.ActivationFunctionType.Sigmoid
                    )
                    # off critical path (runs during the n-tile matmuls):
                    # n = tanh(gi_n + r*gh_n + r*bhh_n) -> c4 = gi_n + r*bhh_n
                    c3 = gpool.tile([128, 4], F32, tag="c3")
                    nc.vector.tensor_mul(c3[:], rz[:, 0:4], bhhn[:])
                    c4 = gpool.tile([128, 4], F32, tag="c4")
                    nc.vector.tensor_add(c4[:], c3[:], gi3[:, t, 8:12])
                    t1 = gpool.tile([128, 4], F32, tag="t1")
                    nc.vector.tensor_mul(t1[:], rz[:, 0:4], gh_n[:])
                    nc.vector.tensor_add(npre[:], t1[:], c4[:])
                    nn_ = gpool.tile([128, 4], F32, tag="nn")
                    nc.scalar.activation(
                        nn_[:], npre[:], mybir.ActivationFunctionType.Tanh
                    )
                    # off-critical-path helpers (overlap with tanh):
                    c1 = gpool.tile([128, 4], F32, tag="c1")  # 1 - z
                    nc.vector.tensor_scalar(
                        c1[:],
                        rz[:, 4:8],
                        -1.0,
                        1.0,
                        mybir.AluOpType.mult,
                        mybir.AluOpType.add,
                    )
                    c2 = gpool.tile([128, 4], F32, tag="c2")  # z * h
                    nc.vector.tensor_mul(
                        c2[:], rz[:, 4:8], ys[:, NH * t : NH * (t + 1)]
                    )
                    t2 = gpool.tile([128, 4], F32, tag="t2")
                    nc.vector.tensor_mul(t2[:], c1[:], nn_[:])
                    # critical path: bf16 state for the next step's matmuls
                    nc.vector.tensor_add(h16[:], t2[:], c2[:])
                    # off critical path: fp32 state archive (read by c2 at t+1)
                    nc.vector.tensor_add(
                        ys[:, NH * (t + 1) : NH * (t + 2)], t2[:], c2[:]
                    )

            nc.sync.dma_start(ys_out[:], ys[:])

    _split_excess_waits(nc)
    return nc


def _prep_weights(conv_w, conv_b, w_ih, w_hh, b_ih, b_hh):
    """Host-side weight layouts (see build_program for the tile maps)."""
    f32 = np.float32
    # wconv[p, ((kc*NM + m)*NJ + j)*128 + co'] = conv_w[128m+co', c, 512j+128s1+p]
    A = conv_w.reshape(NM, 128, CIN, NJ, 4, 128)  # [m, co', c, j, s1, p]
    wconv = np.ascontiguousarray(
        A.transpose(2, 4, 0, 3, 5, 1), dtype=f32
    )  # [c, s1, m, j, p, co']
    wconv = wconv.reshape(NK, NM, NJ, 128, 128).transpose(3, 0, 1, 2, 4).reshape(
        128, NK * NM * NJ * 128
    )
    wconv = np.ascontiguousarray(wconv, dtype=f32)

    # wih[p, (mg*NH + kc)*128 + n'] = w_ih[128mg+n', 128kc+p]
    Aih = w_ih.reshape(NG, 128, NH, 128)  # [mg, n', kc, p]
    wih = np.ascontiguousarray(
        Aih.transpose(3, 0, 2, 1).reshape(128, NG * NH * 128), dtype=f32
    )
    Ahh = w_hh.reshape(NG, 128, NH, 128)
    whh = np.ascontiguousarray(
        Ahh.transpose(3, 0, 2, 1).reshape(128, NG * NH * 128)
    ).astype(ml_dtypes.bfloat16)

    q = (w_ih.astype(np.float64) @ conv_b.astype(np.float64)).astype(f32)
    q = q + b_ih
    q[: 2 * H] = q[: 2 * H] + b_hh[: 2 * H]
    gibias = np.ascontiguousarray(q.reshape(NG, 128).T, dtype=f32)
    bhhn = np.ascontiguousarray(b_hh[2 * H :].reshape(NH, 128).T, dtype=f32)
    return wconv, wih, whh, gibias, bhhn


def kernel(input, hidden, conv_w, conv_b, w_ih, w_hh, b_ih, b_hh, _trace=False):
    input = np.asarray(input, dtype=np.float32)
    hidden = np.asarray(hidden, dtype=np.float32)
    conv_w = np.asarray(conv_w, dtype=np.float32)
    conv_b = np.asarray(conv_b, dtype=np.float32)
    w_ih = np.asarray(w_ih, dtype=np.float32)
    w_hh = np.asarray(w_hh, dtype=np.float32)
    b_ih = np.asarray(b_ih, dtype=np.float32)
    b_hh = np.asarray(b_hh, dtype=np.float32)

    if ("prog", T) not in _CACHE:
        _CACHE[("prog", T)] = build_program(T)
    nc = _CACHE[("prog", T)]

    wconv, wih, whh, gibias, bhhn = _prep_weights(
        conv_w, conv_b, w_ih, w_hh, b_ih, b_hh
    )
    in_maps = []
    for b in range(B):
        xb = input[b].reshape(CIN, NF, STRIDE).transpose(0, 2, 1)
        in_maps.append(
            {
                "x": np.ascontiguousarray(xb),
                "h0": np.ascontiguousarray(hidden[0, b].reshape(NH, 128).T),
                "wconv": wconv,
                "wih": wih,
                "whh": whh,
                "gibias": gibias,
                "bhhn": bhhn,
            }
        )
    res = run_bass_kernel_spmd(nc, in_maps, list(range(B)))

    ys_full = np.empty((T, B, H), dtype=np.float32)
    for b in range(B):
        yb = res.results[b]["ys"]  # [128, NH*(T+1)]
        yb = yb.reshape(128, T + 1, NH)[:, 1:, :]  # drop h0
        ys_full[:, b, :] = yb.transpose(1, 2, 0).reshape(T, H)
    hT = ys_full[-1]  # [B, H]
    out1 = ys_full.reshape(1, T * B, H)
    out2 = hT[None]
    return out1, out2


# revision 18
# speedup vs baseline: 3.6390x; 1.0833x over previous
"""Trainium2 Bass kernel for Conv1d(stride 512, K=3072) -> GRU encoder.

Sharding: data-parallel over batch (B=8) across the 8 NeuronCores; GRU
weights replicated; the sequential time loop runs fully on-device per core.

Per core (batch element b):
  - conv as 6 shifted frame-matmuls, contraction (Cin*512=2048) in 16
    K-chunks of 128, float32r (1 cycle/row moving operand).
  - gi = convT @ w_ih^T + (w_ih@conv_b + b_ih [+ b_hh for r,z]) precomputed
    for all timesteps, laid out [128, T*12] so step t reads [128, 12].
  - GRU scan: per step 48 bf16 matmuls (W_hh stationary tiles [128,128],
    h as [128,1] moving operand) -> gh^T in PSUM [128, 12] where column m
    holds neurons [128m, 128m+128); gates on DVE/ACT; h state kept fp32 in
    the ys buffer, with a bf16 copy feeding the next step's matmuls.
"""
import os
import sys

sys.path.insert(0, "/opt/trn_rl_repo")

import numpy as np
import ml_dtypes

import concourse.bass as bass
import concourse.tile as tile
from concourse import mybir
from concourse.bass_utils import run_bass_kernel_spmd
from concourse.vector_clock import ScopedClock

# ---------------------------------------------------------------- constants
B, CIN, L = 8, 4, 524288
KERNEL, STRIDE = 3072, 512
COUT, H = 512, 512
G3 = 3 * H  # 1536
NF = L // STRIDE  # 1024 frames
T = (L - KERNEL) // STRIDE + 1  # 1019
NK = (CIN * STRIDE) // 128  # 16 contraction chunks
NJ = KERNEL // STRIDE  # 6 shifts
NM = COUT // 128  # 4 conv out tiles
NG = G3 // 128  # 12 gate tiles
NH = H // 128  # 4 h chunks

F32 = mybir.dt.float32
F32R = mybir.dt.float32r
BF16 = mybir.dt.bfloat16

_CACHE = {}

# timing experiments: "full" (default), "mm" (scan matmuls only),
# "gates" (gate ops only, no matmuls)
SCAN_MODE = os.environ.get("SCAN_MODE", "full")


def _patch_tile_drain():
    """This walrus build rejects >2 (CTRL: >1) sync waits per instruction;
    spread the Tile kernel-tail drain waits over single-wait SP nops."""
    if getattr(tile.TileContext, "_drain_patched", False):
        return

    def _drain_and_barrier(self, tick_clock, wait_clock):
        nop_inst = self.nc.sync.nop()
        wait_clock.add_sem_waits(
            nop_inst.ins, ScopedClock({None: tick_clock.global_clock})
        )
        si = nop_inst.ins.sync_info
        if si is not None and si.on_wait and len(si.on_wait) > 1:
            waits = list(si.on_wait)
            updates = list(si.on_update)
            nop_inst.ins.sync_info = mybir.SyncInfo(
                on_wait=waits[:1], on_update=updates
            )
            for i in range(1, len(waits)):
                extra = self.nc.sync.nop()
                extra.ins.sync_info = mybir.SyncInfo(
                    on_wait=waits[i : i + 1], on_update=[]
                )
        self.nc.sync.drain()
        self.nc.all_engine_barrier()
        assert self.sems is not None
        popped = self.nc._tile_sem_poison_stack.pop()
        assert popped is self._sem_poison
        self.nc.clear_and_free_semaphores(list(self.sems.allocated().values()))
        self.nc.all_engine_barrier()

    tile.TileContext._drain_and_barrier = _drain_and_barrier
    tile.TileContext._drain_patched = True


_CTRL_OPS = {"NoOp", "Drain"}


def _split_excess_waits(nc):
    """This walrus build allows at most 2 sync waits per instruction (1 for
    CTRL ops like NoOp/Drain). Hoist excess waits onto preceding same-engine
    single-wait nops."""
    for fn in nc.m.functions:
        for blk in fn.blocks:
            changed = False
            out = []
            for inst in blk.instructions:
                si = inst.sync_info
                limit = 1
                if si is not None and si.on_wait and len(si.on_wait) > limit:
                    waits = list(si.on_wait)
                    extra, keep = waits[:-limit], waits[-limit:]
                    for k, w in enumerate(extra):
                        out.append(
                            mybir.InstNoOp(
                                name=f"{inst.name}_ws{k}",
                                engine=inst.engine,
                                sync_info=mybir.SyncInfo(on_wait=[w], on_update=[]),
                            )
                        )
                    inst.sync_info = mybir.SyncInfo(
                        on_wait=keep, on_update=list(si.on_update)
                    )
                    changed = True
                out.append(inst)
            if changed:
                blk.instructions = out


def build_program(t_steps=T):
    """Build the single-core Bass program (same program runs on all 8 cores
    with per-core input data)."""
    _patch_tile_drain()
    nc = bass.Bass("TRN2", target_bir_lowering=False, debug=False)

    x_in = nc.dram_tensor("x", [CIN, STRIDE, NF], F32R, kind="ExternalInput").ap()
    h0_in = nc.dram_tensor("h0", [128, NH], F32, kind="ExternalInput").ap()
    wconv_in = nc.dram_tensor(
        "wconv", [128, NK * NM * NJ * 128], F32R, kind="ExternalInput"
    ).ap()
    wih_in = nc.dram_tensor("wih", [128, NG * NH * 128], F32R, kind="ExternalInput").ap()
    whh_in = nc.dram_tensor(
        "whh", [128, NG * NH * 128], BF16, kind="ExternalInput"
    ).ap()
    gibias_in = nc.dram_tensor("gibias", [128, NG], F32, kind="ExternalInput").ap()
    bhhn_in = nc.dram_tensor("bhhn", [128, NH], F32, kind="ExternalInput").ap()
    ys_out = nc.dram_tensor(
        "ys", [128, NH * (t_steps + 1)], F32, kind="ExternalOutput"
    ).ap()

    with tile.TileContext(nc) as tc:
        with (
            tc.tile_pool(name="consts", bufs=1) as cpool,
            tc.tile_pool(name="wih", bufs=1) as wihpool,
            tc.tile_pool(name="whh", bufs=1) as whhpool,
            tc.tile_pool(name="convT", bufs=1) as ctpool,
            tc.tile_pool(name="gi", bufs=1) as gipool,
            tc.tile_pool(name="ys", bufs=1) as yspool,
            tc.tile_pool(name="hbf", bufs=1) as hpool,
            tc.tile_pool(name="gates", bufs=2) as gpool,
        ):
            # ---- persistent tiles
            gibias = cpool.tile([128, NG], F32, tag="gibias")
            nc.sync.dma_start(gibias[:], gibias_in[:])
            bhhn = cpool.tile([128, NH], F32, tag="bhhn")
            nc.sync.dma_start(bhhn[:], bhhn_in[:])
            wih = wihpool.tile([128, NG * NH * 128], F32R)
            nc.sync.dma_start(wih[:], wih_in[:])
            whh = whhpool.tile([128, NG * NH * 128], BF16)
            nc.sync.dma_start(whh[:], whh_in[:])
            convT = ctpool.tile([128, NM * NF], F32R)
            gi = gipool.tile([128, t_steps * NG], F32)
            ys = yspool.tile([128, NH * (t_steps + 1)], F32)
            h16 = hpool.tile([128, NH], BF16)

            nc.sync.dma_start(ys[:, 0:NH], h0_in[:])
            nc.vector.tensor_copy(h16[:], ys[:, 0:NH])

            # ---- conv phase: psum[m, tc] accumulates over all (kc, j)
            # moving free dims must be even for fp32r; pad odd chunks by one
            # (padded frames are zeroed; padded outputs are discarded)
            if t_steps > 512:
                nts = [512, t_steps - 512]
            else:
                nts = [t_steps]
            mm_nts = [n + (n % 2) for n in nts]
            NFP = NF + 8
            conv_scope = tc.tile_pool(name="inp", bufs=2)
            inppool = conv_scope.__enter__()
            wconv_scope = tc.tile_pool(name="wconv", bufs=2)
            wcpool = wconv_scope.__enter__()
            psconv_scope = tc.tile_pool(name="psconv", bufs=1, space="PSUM")
            psconv = psconv_scope.__enter__()
            ps_conv = [
                [psconv.tile([128, 512], F32, tag=f"psc{m}_{tci}", name=f"psc{m}_{tci}") for tci in range(len(nts))]
                for m in range(NM)
            ]
            for kc in range(NK):
                inp_k = inppool.tile([128, NFP], F32R, tag="inp")
                nc.vector.memset(inp_k[:, NF:NFP].bitcast(mybir.dt.uint32), 0)
                # x[c] viewed [NF, 512]; partitions = s-chunk, free = frame
                c, s1 = kc // 4, kc % 4
                nc.sync.dma_start(
                    inp_k[:, :NF], x_in[c, 128 * s1 : 128 * (s1 + 1), :]
                )
                wconv_k = wcpool.tile([128, NM * NJ * 128], F32R, tag="wconv")
                nc.sync.dma_start(
                    wconv_k[:],
                    wconv_in[:, kc * NM * NJ * 128 : (kc + 1) * NM * NJ * 128],
                )
                for m in range(NM):
                    for j in range(NJ):
                        lhsT = wconv_k[:, (m * NJ + j) * 128 : (m * NJ + j + 1) * 128]
                        for tci, nt in enumerate(mm_nts):
                            nc.tensor.matmul(
                                ps_conv[m][tci][:, :nt],
                                lhsT,
                                inp_k[:, j + 512 * tci : j + 512 * tci + nt],
                                start=(kc == 0 and j == 0),
                                stop=(kc == NK - 1 and j == NJ - 1),
                            )
            # evacuate conv psum -> convT (column layout: m*NF + t)
            for m in range(NM):
                for tci, nt in enumerate(nts):
                    nc.vector.tensor_copy(
                        convT[:, m * NF + 512 * tci : m * NF + 512 * tci + nt],
                        ps_conv[m][tci][:, :nt],
                    )

            psconv_scope.__exit__(None, None, None)
            wconv_scope.__exit__(None, None, None)
            conv_scope.__exit__(None, None, None)

            # ---- gi phase: gi^T = wih @ convT (+bias), layout [128, t*NG+g]
            gi3 = gi[:].rearrange("p (t g) -> p t g", g=NG)
            psgi_scope = tc.tile_pool(name="psgi", bufs=2, space="PSUM")
            psgi = psgi_scope.__enter__()
            for mg in range(NG):
                for tci, nt in enumerate(nts):
                    ntm = mm_nts[tci]
                    ps = psgi.tile([128, 512], F32, tag="psgi")
                    for kc in range(NH):
                        nc.tensor.matmul(
                            ps[:, :ntm],
                            wih[:, (mg * NH + kc) * 128 : (mg * NH + kc + 1) * 128],
                            convT[:, kc * NF + 512 * tci : kc * NF + 512 * tci + ntm],
                            start=(kc == 0),
                            stop=(kc == NH - 1),
                        )
                    nc.vector.tensor_scalar(
                        gi3[:, 512 * tci : 512 * tci + nt, mg],
                        ps[:, :nt],
                        gibias[:, mg : mg + 1],
                        None,
                        mybir.AluOpType.add,
                    )

            psgi_scope.__exit__(None, None, None)

            # ---- GRU scan
            with tc.tile_pool(name="psgh", bufs=2, space="PSUM") as psgh:
                for t in range(t_steps):
                    gh_rz = psgh.tile([128, 8], F32, tag="ghrz")
                    gh_n = psgh.tile([128, NH], F32, tag="ghn")
                    prerz = psgh.tile([128, 8], F32, tag="prz")
                    npre = psgh.tile([128, 4], F32, tag="npr")
                    for mg in range(NG if SCAN_MODE != "gates" else 0):
                        dst = (
                            gh_rz[:, mg : mg + 1]
                            if mg < 8
                            else gh_n[:, mg - 8 : mg - 7]
                        )
                        for kc in range(NH):
                            nc.tensor.matmul(
                                dst,
                                whh[:, (mg * NH + kc) * 128 : (mg * NH + kc + 1) * 128],
                                h16[:, kc : kc + 1],
                                start=(kc == 0),
                                stop=(kc == NH - 1),
                            )
                    if SCAN_MODE == "mm":
                        # keep only the h16 update so steps stay serialized
                        nc.vector.tensor_copy(h16[:], ys[:, 0:NH])
                        continue
                    if SCAN_MODE == "gates":
                        # fake the psum tiles so gates read something
                        nc.vector.memset(gh_rz[:], 0.01)
                        nc.vector.memset(gh_n[:], 0.01)
                    nc.vector.tensor_add(prerz[:], gh_rz[:], gi3[:, t, 0:8])
                    rz = gpool.tile([128, 8], F32, tag="rz")
                    nc.scalar.activation(
                        rz[:], prerz[:], mybir.ActivationFunctionType.Sigmoid
                    )
                    # off critical path (runs during the n-tile matmuls):
                    # n = tanh(gi_n + r*gh_n + r*bhh_n) -> c4 = gi_n + r*bhh_n
                    c3 = gpool.tile([128, 4], F32, tag="c3")
                    nc.vector.tensor_mul(c3[:], rz[:, 0:4], bhhn[:])
                    c4 = gpool.tile([128, 4], F32, tag="c4")
                    nc.vector.tensor_add(c4[:], c3[:], gi3[:, t, 8:12])
                    t1 = gpool.tile([128, 4], F32, tag="t1")
                    nc.vector.tensor_mul(t1[:], rz[:, 0:4], gh_n[:])
                    nc.vector.tensor_add(npre[:], t1[:], c4[:])
                    nn_ = gpool.tile([128, 4], F32, tag="nn")
                    nc.scalar.activation(
                        nn_[:], npre[:], mybir.ActivationFunctionType.Tanh
                    )
                    # off-critical-path helpers (overlap with tanh):
                    c1 = gpool.tile([128, 4], F32, tag="c1")  # 1 - z
                    nc.vector.tensor_scalar(
                        c1[:],
                        rz[:, 4:8],
                        -1.0,
                        1.0,
                        mybir.AluOpType.mult,
                        mybir.AluOpType.add,
                    )
                    c2 = gpool.tile([128, 4], F32, tag="c2")  # z * h
                    nc.vector.tensor_mul(
                        c2[:], rz[:, 4:8], ys[:, NH * t : NH * (t + 1)]
                    )
                    t2 = gpool.tile([128, 4], F32, tag="t2")
                    nc.vector.tensor_mul(t2[:], c1[:], nn_[:])
                    # critical path: bf16 state for the next step's matmuls
                    nc.vector.tensor_add(h16[:], t2[:], c2[:])
                    # off critical path: fp32 state archive (read by c2 at t+1)
                    nc.vector.tensor_add(
                        ys[:, NH * (t + 1) : NH * (t + 2)], t2[:], c2[:]
                    )

            nc.sync.dma_start(ys_out[:], ys[:])

    _split_excess_waits(nc)
    return nc


def _prep_weights(conv_w, conv_b, w_ih, w_hh, b_ih, b_hh):
    """Host-side weight layouts (see build_program for the tile maps)."""
    f32 = np.float32
    # wconv[p, ((kc*NM + m)*NJ + j)*128 + co'] = conv_w[128m+co', c, 512j+128s1+p]
    A = conv_w.reshape(NM, 128, CIN, NJ, 4, 128)  # [m, co', c, j, s1, p]
    wconv = np.ascontiguousarray(
        A.transpose(2, 4, 0, 3, 5, 1), dtype=f32
    )  # [c, s1, m, j, p, co']
    wconv = wconv.reshape(NK, NM, NJ, 128, 128).transpose(3, 0, 1, 2, 4).reshape(
        128, NK * NM * NJ * 128
    )
    wconv = np.ascontiguousarray(wconv, dtype=f32)

    # wih[p, (mg*NH + kc)*128 + n'] = w_ih[128mg+n', 128kc+p]
    Aih = w_ih.reshape(NG, 128, NH, 128)  # [mg, n', kc, p]
    wih = np.ascontiguousarray(
        Aih.transpose(3, 0, 2, 1).reshape(128, NG * NH * 128), dtype=f32
    )
    Ahh = w_hh.reshape(NG, 128, NH, 128)
    whh = np.ascontiguousarray(
        Ahh.transpose(3, 0, 2, 1).reshape(128, NG * NH * 128)
    ).astype(ml_dtypes.bfloat16)

    q = (w_ih.astype(np.float64) @ conv_b.astype(np.float64)).astype(f32)
    q = q + b_ih
    q[: 2 * H] = q[: 2 * H] + b_hh[: 2 * H]
    gibias = np.ascontiguousarray(q.reshape(NG, 128).T, dtype=f32)
    bhhn = np.ascontiguousarray(b_hh[2 * H :].reshape(NH, 128).T, dtype=f32)
    return wconv, wih, whh, gibias, bhhn


def kernel(input, hidden, conv_w, conv_b, w_ih, w_hh, b_ih, b_hh, _trace=False):
    input = np.asarray(input, dtype=np.float32)
    hidden = np.asarray(hidden, dtype=np.float32)
    conv_w = np.asarray(conv_w, dtype=np.float32)
    conv_b = np.asarray(conv_b, dtype=np.float32)
    w_ih = np.asarray(w_ih, dtype=np.float32)
    w_hh = np.asarray(w_hh, dtype=np.float32)
    b_ih = np.asarray(b_ih, dtype=np.float32)
    b_hh = np.asarray(b_hh, dtype=np.float32)

    if ("prog", T) not in _CACHE:
        _CACHE[("prog", T)] = build_program(T)
    nc = _CACHE[("prog", T)]

    wconv, wih, whh, gibias, bhhn = _prep_weights(
        conv_w, conv_b, w_ih, w_hh, b_ih, b_hh
    )
    in_maps = []
    for b in range(B):
        xb = input[b].reshape(CIN, NF, STRIDE).transpose(0, 2, 1)
        in_maps.append(
            {
                "x": np.ascontiguousarray(xb),
                "h0": np.ascontiguousarray(hidden[0, b].reshape(NH, 128).T),
                "wconv": wconv,
                "wih": wih,
                "whh": whh,
                "gibias": gibias,
                "bhhn": bhhn,
            }
        )
    res = run_bass_kernel_spmd(nc, in_maps, list(range(B)))

    ys_full = np.empty((T, B, H), dtype=np.float32)
    for b in range(B):
        yb = res.results[b]["ys"]  # [128, NH*(T+1)]
        yb = yb.reshape(128, T + 1, NH)[:, 1:, :]  # drop h0
        ys_full[:, b, :] = yb.transpose(1, 2, 0).reshape(T, H)
    hT = ys_full[-1]  # [B, H]
    out1 = ys_full.reshape(1, T * B, H)
    out2 = hT[None]
    return out1, out2
